# revision 3
# baseline (speedup 1.0000x reference)
"""AttentionPooling Trainium2 kernel, v2.

Math (per batch row b):
    x   = target[b] + hist[b]              # [T, D]
    h   = relu(x @ W + Wb)                 # [T, D]
    lg  = h @ q  (+ q_bias, softmax-invariant -> ignored)
    s   = softmax(lg) over T
    out = sum_t s_t * hist[b, t]           # [D]

v2 design (pure data parallel over batch across 8 cores):
  - T2 "parity" layout: t = 2*t2 + par; hist loaded HBM->SBUF with
    fp32->bf16 cast (SWDGE) into nt [t2=100 part, (b, par, d)] — the
    (par, d) pairs are 1024B-contiguous in HBM, halving descriptor count
    vs the 512B [t,(b,d)] layout (measured ~571us vs ~717us per exec;
    2KB descriptors with cast collapse, so T2 is the sweet spot).
  - PE transposes nt -> xT [d, (b, par, t2)] fused with broadcast-add of
    targetT on DVE (psum drain).
  - Main matmul: H^T = W^T @ xT (bf16, W stationary). The relu+bias psum
    drains rotate ACT/ACT/DVE ("aad") — a lone ACT (~830ns per 512-col
    activation on HW) would be the bottleneck engine.
  - Logits via q replicated 32x (stationary q32): per 4-b group, 200-col
    matmuls land logits [32-replicated rows, t]; one exp per group-pair
    on ACT.
  - w transposed back (PE) to [t2, 32-replica cols] for use as the
    pooling stationary (psum->sbuf staging on DVE).
  - Pooling matmul per b accumulates par0+par1 into psum rows 32*(b%4);
    per-pair wsum via one ones-stationary matmul over 32-strided replica
    columns.  Drains alternate DVE/ACT; bf16 out_dev halves the output
    traffic (PJRT output handling is a large fixed cost per exec).
  - Final normalize (divide by wsum) + layout de-permute on host.
"""

import sys

sys.path.insert(0, "/opt/trn_rl_repo")

import numpy as np

import concourse.bacc as bacc
import concourse.bass as bass
import concourse.mybir as mybir
import concourse.tile as tile
from concourse import masks
from concourse.bass_utils import run_bass_kernel_spmd

F32 = mybir.dt.float32
BF16 = mybir.dt.bfloat16
AF = mybir.ActivationFunctionType

NCORES = 8
B, T, D = 16384, 200, 128
BC = B // NCORES          # 2048 batch rows per core
T2 = T // 2               # 100 t2 partitions, 2 parities
E1 = D + 1                # d cols + ones col
B_IT = 64                 # batch rows per outer iteration
NSUB = B_IT // 4          # 16 sub-blocks of 4 b's (transposes)
NGRP = B_IT // 4          # 16 groups of 4 b's (pool)
GW = 2 * D + 16              # per-g2 out cols: 2x128 pooled + 16 wsums
OUTW = (NGRP // 2) * GW      # 2176 out cols per iter


def build(nc, b_core=BC):
    nit = b_core // B_IT
    hist = nc.dram_tensor("hist", [b_core, T, D], F32, kind="ExternalInput")
    tgt = nc.dram_tensor("target", [b_core, D], F32, kind="ExternalInput")
    w_in = nc.dram_tensor("W", [D, D], F32, kind="ExternalInput")
    wb_in = nc.dram_tensor("Wb", [D], F32, kind="ExternalInput")
    q_in = nc.dram_tensor("q", [D, 1], F32, kind="ExternalInput")
    out_dev = nc.dram_tensor("out_dev", [nit, 4, OUTW], BF16, kind="ExternalOutput")

    from contextlib import ExitStack
    with tile.TileContext(nc) as tc, ExitStack() as es:
        consts = es.enter_context(tc.tile_pool(name="consts", bufs=1))
        nt_pool = es.enter_context(tc.tile_pool(name="nt", bufs=1))
        ht_pool = es.enter_context(tc.tile_pool(name="ht", bufs=CFG["ht"]))
        h_pool = es.enter_context(tc.tile_pool(name="h", bufs=CFG["hh"]))
        w_pool = es.enter_context(tc.tile_pool(name="w", bufs=CFG["w"]))
        out_pool = es.enter_context(tc.tile_pool(name="out", bufs=CFG["outt"]))
        ps_tp = es.enter_context(tc.tile_pool(name="ps_tp", bufs=CFG["tp"], space="PSUM"))
        ps_mm = es.enter_context(tc.tile_pool(name="ps_mm", bufs=CFG["mm"], space="PSUM"))
        ps_q = es.enter_context(tc.tile_pool(name="ps_q", bufs=CFG["q"], space="PSUM"))
        ps_pool = es.enter_context(tc.tile_pool(name="ps_pool", bufs=CFG["pool"], space="PSUM"))

        # ---- constants ----
        ident = consts.tile([128, 128], BF16)
        masks.make_identity(nc, ident[:, :])

        w_f32 = consts.tile([D, D], F32)
        nc.sync.dma_start(out=w_f32, in_=w_in.ap())
        w_bf = consts.tile([D, D], BF16)
        nc.vector.tensor_copy(out=w_bf, in_=w_f32)

        wbias = consts.tile([D, 1], F32)
        nc.sync.dma_start(out=wbias, in_=wb_in.ap()[:, None])

        q_f32 = consts.tile([D, 1], F32)
        nc.sync.dma_start(out=q_f32, in_=q_in.ap())
        q_bf = consts.tile([D, 1], BF16)
        nc.vector.tensor_copy(out=q_bf, in_=q_f32)
        q32 = consts.tile([D, 32], BF16)
        nc.vector.tensor_copy(
            out=q32,
            in_=bass.AP(tensor=q_bf.tensor, offset=q_bf.offset,
                        ap=[q_bf.ap[0], [0, 32]]),
        )

        # targetT [d, b_core] bf16
        tgtT = consts.tile([D, b_core], BF16)
        for k in range((b_core + 127) // 128):
            bn = min(128, b_core - k * 128)
            t_f32 = w_pool.tile([128, D], F32, tag="tsetup", bufs=2)
            nc.sync.dma_start(out=t_f32[0:bn], in_=tgt.ap()[k * 128:k * 128 + bn, :])
            t_bf = w_pool.tile([128, D], BF16, tag="tsetup_bf", bufs=2)
            nc.vector.tensor_copy(out=t_bf[0:bn], in_=t_f32[0:bn])
            tp = ps_tp.tile([128, 4 * T], BF16, tag="tp")
            nc.tensor.transpose(tp[:, 0:bn], t_bf[0:bn], ident[0:bn, 0:bn])
            nc.vector.tensor_copy(out=tgtT[:, k * 128:k * 128 + bn], in_=tp[:, 0:bn])

        # ones block for the per-g2 wsum matmul (all 128 out rows written
        # so the psum drain copy reads fully-initialized data)
        ones128 = consts.tile([T2, 128], BF16)
        nc.vector.memset(ones128, 1.0)

        # persistent nt buffers: [t2, (b, par, d)] — (par, d) contiguous in
        # HBM so the cast DMA gets 1024B descriptors
        nt_bufs = []
        for nb in range(2):
            ntb = nt_pool.tile([T2, B_IT * 2 * D], BF16, tag=f"nt{nb}")
            nt_bufs.append(ntb)

        # ---- main loop ----
        for it in range(nit):
            b0 = it * B_IT
            nt = nt_bufs[it % 2]
            ntv = nt.rearrange("t (b p e) -> t b p e", p=2, e=D)
            bs = B_IT // 2
            for s in range(2):
                if CFG.get("tiny_dma"):
                    nc.gpsimd.dma_start(
                        out=ntv[0:1, s * bs:s * bs + 1, 0, 0:D],
                        in_=hist.ap()[b0:b0 + 1, 0:1, :]
                        .rearrange("b t d -> t b d"))
                    continue
                nc.gpsimd.dma_start(
                    out=ntv[:, s * bs:(s + 1) * bs, :, :].rearrange(
                        "t b p e -> t b (p e)"),
                    in_=hist.ap()[b0 + s * bs:b0 + (s + 1) * bs, :, :]
                    .rearrange("b (t x) d -> t b (x d)", x=2),
                )

            # targetT expanded 8x along t for an aligned broadcast-add AP
            tgx = w_pool.tile([128, B_IT * 8], BF16, tag="tgx")
            sl = tgtT[:, b0:b0 + B_IT]
            nc.vector.tensor_copy(
                out=tgx,
                in_=bass.AP(tensor=sl.tensor, offset=sl.offset,
                            ap=[sl.ap[0], sl.ap[1], [0, 8]]),
            )
            tgxv = tgx.rearrange("d (b r) -> d b r", r=8)

            # histT + targetT broadcast -> xT [d, (b, par, t2)]
            ht = ht_pool.tile([128, B_IT * T], BF16, tag="ht")
            htv = ht.rearrange("d (b t) -> d b t", t=T)
            for m in range(NSUB) if "tp" not in SKIP else []:
                tp = ps_tp.tile([128, 4 * T], BF16, tag="tp")
                tpv = tp.rearrange("d (b t) -> d b t", t=T)
                for bl in range(4):
                    bb = 4 * m + bl
                    for par in range(2):
                        nc.tensor.transpose(
                            tpv[:, bl, par * T2:(par + 1) * T2],
                            ntv[:, bb, par, 0:D],
                            ident[0:T2, 0:T2])
                hts = htv[:, 4 * m:4 * m + 4, :]
                tg4 = tgxv[:, 4 * m:4 * m + 4, :]
                nc.vector.tensor_add(
                    hts.rearrange("d b (to ti) -> d b to ti", ti=8),
                    tp.rearrange("d (b to ti) -> d b to ti", b=4, ti=8),
                    bass.AP(tensor=tg4.tensor, offset=tg4.offset,
                            ap=[tg4.ap[0], tg4.ap[1], [0, T // 8], tg4.ap[2]]),
                )

            # H^T = relu(W^T xT + bias)  [e, (b, par, t2)]
            # psum drains rotate across ACT / DVE / GPSIMD: the ACT engine
            # alone (~830ns per 512-col activation) would be the bottleneck
            hh = h_pool.tile([128, B_IT * T], BF16, tag="hh")
            nmm = (B_IT * T) // 512
            drain_plan = CFG.get("drain", "aad")
            for k in range(nmm) if "mm" not in SKIP else []:
                mm = ps_mm.tile([128, 512], F32, tag="mm")
                nc.tensor.matmul(mm, w_bf, ht[:, 512 * k:512 * (k + 1)],
                                 start=True, stop=True)
                eng = drain_plan[k % len(drain_plan)]
                dst = hh[:, 512 * k:512 * (k + 1)]
                if eng == "a":
                    nc.scalar.activation(dst, mm, AF.Relu, bias=wbias)
                elif eng == "d":
                    nc.vector.tensor_scalar(
                        dst, mm, wbias, 0.0,
                        mybir.AluOpType.add, mybir.AluOpType.max)
                else:
                    nc.gpsimd.tensor_scalar(
                        dst, mm, wbias, 0.0,
                        mybir.AluOpType.add, mybir.AluOpType.max)

            # logits via q32 (wide moving, 32-replicated rows), exp on ACT;
            # wtile [32-repl rows, (g-pair, t)] in (par, t2) order
            hv = hh.rearrange("e (b t) -> e b t", t=T)
            wtiles = {}
            if "q" not in SKIP:
                for gp in range(NGRP // 2):
                    qp = ps_q.tile([128, 2 * T], F32, tag="lg")
                    for gg in range(2):
                        g = 2 * gp + gg
                        for j in range(4):
                            nc.tensor.matmul(
                                qp[32 * j:32 * j + 32,
                                   gg * T:(gg + 1) * T],
                                q32, hv[:, 4 * g + j, :],
                                start=True, stop=True,
                                skip_group_check=True,
                                tile_position=(0, 32 * j))
                    wtile = w_pool.tile([128, 2 * T], BF16, tag="wtile")
                    nc.scalar.activation(wtile, qp, AF.Exp)
                    wtiles[gp] = wtile

            # pooling: per g2 transpose w to [t2, cols] then accumulate pars
            outt = out_pool.tile([128, OUTW], BF16, tag="outt")
            for g2 in range(NGRP // 2) if "pool" not in SKIP else []:
                wtile = wtiles[g2]
                wt_ps = ps_tp.tile([T2, 512], BF16, tag="tp")
                for gg in range(2):
                    for par in range(2):
                        nc.tensor.transpose(
                            wt_ps[:, (2 * gg + par) * 128:
                                  (2 * gg + par) * 128 + 128],
                            wtile[:, gg * T + par * T2:
                                  gg * T + par * T2 + T2],
                            ident)
                wt_sb = w_pool.tile([T2, 512], BF16, tag="wt_sb")
                nc.vector.tensor_copy(out=wt_sb, in_=wt_ps)
                pp = ps_pool.tile([128, GW], F32, tag="pp")
                for gg in range(2):
                    g = 2 * g2 + gg
                    for j in range(4):
                        bb = 4 * g + j

                        def st32(par):
                            return wt_sb[:, (2 * gg + par) * 128 + 32 * j:
                                         (2 * gg + par) * 128 + 32 * j + 32]

                        nc.tensor.matmul(
                            pp[32 * j:32 * j + 32, D * gg:D * (gg + 1)],
                            st32(0), ntv[:, bb, 0, :],
                            start=True, stop=False,
                            skip_group_check=True,
                            tile_position=(0, 32 * j))
                        nc.tensor.matmul(
                            pp[32 * j:32 * j + 32, D * gg:D * (gg + 1)],
                            st32(1), ntv[:, bb, 1, :],
                            start=False, stop=True,
                            skip_group_check=True,
                            tile_position=(0, 32 * j))
                # wsum: column sums of wt_sb's replica columns, one col per
                # (gg, par, j); all 128 psum rows written via ones block
                wssl = wt_sb[:, 0:512]
                nc.tensor.matmul(
                    pp[:, 2 * D:GW],
                    ones128,
                    bass.AP(tensor=wssl.tensor, offset=wssl.offset,
                            ap=[wssl.ap[0], [32, 16]]),
                    start=True, stop=True, skip_group_check=True)
                if g2 % 2 == 0:
                    nc.vector.tensor_copy(
                        out=outt[:, GW * g2:GW * (g2 + 1)], in_=pp)
                else:
                    nc.scalar.activation(
                        outt[:, GW * g2:GW * (g2 + 1)], pp, AF.Copy)

            for j in range(4) if "pool" not in SKIP else []:
                nc.sync.dma_start(
                    out=out_dev.ap()[it, j, :],
                    in_=outt[32 * j:32 * j + 1, :],
                )

    return out_dev


def decode_out(arr, b_core=BC):
    """[nit, 4, OUTW] bf16 -> pooled [b_core, D], wsum [b_core]."""
    nit = b_core // B_IT
    a = np.asarray(arr).astype(np.float32).reshape(nit, 4, NGRP // 2, GW)
    p = a[..., 0:2 * D].reshape(nit, 4, NGRP // 2, 2, D)
    p = np.transpose(p, (0, 2, 3, 1, 4)).reshape(b_core, D)
    w = a[..., 2 * D:GW].reshape(nit, 4, NGRP // 2, 2, 2, 4)
    idx = np.arange(4)
    # rows 32j all hold the same sums; take row j for column j
    w = w[:, idx, :, :, :, idx]              # [4(j), nit, 8(g2), 2(gg), 2(par)]
    w = w.sum(axis=4)                        # sum parities
    w = np.transpose(w, (1, 2, 3, 0)).reshape(b_core)
    return p, w


_cache = {}
LAST_RESULT = None
SKIP = set()
CFG = dict(tp=2, mm=2, q=2, pool=2, ht=2, hh=1, outt=2, w=2)


def _get_program(b_core):
    key = (b_core, tuple(sorted(SKIP)), tuple(sorted(CFG.items())))
    if key not in _cache:
        nc = bacc.Bacc("TRN2", target_bir_lowering=False, debug=False,
                       num_devices=NCORES)
        build(nc, b_core)
        nc.compile()
        _cache[key] = nc
    return _cache[key]


def kernel(**inputs):
    hist = np.ascontiguousarray(np.asarray(inputs["hist_embeddings"], np.float32))
    tgt = np.ascontiguousarray(np.asarray(inputs["target_embedding"], np.float32))
    W = np.ascontiguousarray(np.asarray(inputs["W_kernel"], np.float32))
    Wb = np.ascontiguousarray(np.asarray(inputs["W_bias"], np.float32))
    q = np.ascontiguousarray(np.asarray(inputs["q_kernel"], np.float32))
    # q_bias shifts every logit equally -> softmax-invariant -> ignored.

    nc = _get_program(BC)
    in_maps = []
    for c in range(NCORES):
        sl = slice(c * BC, (c + 1) * BC)
        in_maps.append({
            "hist": hist[sl], "target": tgt[sl],
            "W": W, "Wb": Wb, "q": q,
        })
    res = run_bass_kernel_spmd(nc, in_maps, core_ids=list(range(NCORES)))
    global LAST_RESULT
    LAST_RESULT = res
    outs = []
    for c in range(NCORES):
        pooled, wsum = decode_out(res.results[c]["out_dev"])
        outs.append(pooled / wsum[:, None])
    return np.concatenate(outs, axis=0).astype(np.float32)


def timed_run(inputs, iters=5, bcs=BC):
    """Device-resident repeated execution; returns (best_seconds, outputs)."""
    import time
    import jax
    from jax.sharding import Mesh, PartitionSpec
    from jax.experimental.shard_map import shard_map
    import concourse.mybir as mybir_
    from concourse.bass2jax import (install_neuronx_cc_hook, _bass_exec_p,
                                    partition_id_tensor)

    hist = np.ascontiguousarray(np.asarray(inputs["hist_embeddings"], np.float32))
    tgt = np.ascontiguousarray(np.asarray(inputs["target_embedding"], np.float32))
    W = np.ascontiguousarray(np.asarray(inputs["W_kernel"], np.float32))
    Wb = np.ascontiguousarray(np.asarray(inputs["W_bias"], np.float32))
    q = np.ascontiguousarray(np.asarray(inputs["q_kernel"], np.float32))
    hist = hist[:NCORES * bcs]
    tgt = tgt[:NCORES * bcs]
    nc = _get_program(bcs)
    install_neuronx_cc_hook()

    pid_name = nc.partition_id_tensor.name if nc.partition_id_tensor else None
    in_names, out_names, out_avals, zero_outs = [], [], [], []
    for alloc in nc.m.functions[0].allocations:
        if not isinstance(alloc, mybir_.MemoryLocationSet):
            continue
        name = alloc.memorylocations[0].name
        if alloc.kind == "ExternalInput":
            if name != pid_name:
                in_names.append(name)
        elif alloc.kind == "ExternalOutput":
            shape = tuple(alloc.tensor_shape)
            dtype = mybir_.dt.np(alloc.dtype)
            out_names.append(name)
            out_avals.append(jax.core.ShapedArray(shape, dtype))
            zero_outs.append(np.zeros(shape, dtype))
    all_names = in_names + out_names
    if pid_name is not None:
        all_names = all_names + [pid_name]

    import os
    chain = int(os.environ.get("KERNEL_CHAIN", "1"))

    aliases = tuple((oi, len(in_names) + oi) for oi in range(len(out_names)))

    def _body(*args):
        nin_ = len(in_names)
        ins_ = list(args[:nin_])
        outs = list(args[nin_:])
        for _ in range(chain):
            operands = ins_ + outs
            if pid_name is not None:
                operands = operands + [partition_id_tensor()]
            outs = list(_bass_exec_p.bind(
                *operands, out_avals=tuple(out_avals),
                in_names=tuple(all_names), out_names=tuple(out_names),
                lowering_input_output_aliases=aliases,
                sim_require_finite=True, sim_require_nnan=True, nc=nc))
        return tuple(outs)

    devices = jax.devices()[:NCORES]
    mesh = Mesh(np.array(devices), ("core",))
    nin = len(in_names) + len(out_names)
    fn = jax.jit(shard_map(_body, mesh=mesh,
                           in_specs=(PartitionSpec("core"),) * nin,
                           out_specs=(PartitionSpec("core"),) * len(out_names),
                           check_rep=False),
                 donate_argnums=tuple(range(len(in_names), nin)))
    full = {"hist": hist, "target": tgt,
            "W": np.concatenate([W] * NCORES, 0),
            "Wb": np.concatenate([Wb] * NCORES, 0),
            "q": np.concatenate([q] * NCORES, 0)}
    args = [full[n] for n in in_names] + [
        np.concatenate([z] * NCORES, 0) for z in zero_outs]
    sh = jax.sharding.NamedSharding(mesh, PartitionSpec("core"))
    dargs = [jax.device_put(a, sh) for a in args]
    r = fn(*dargs)
    jax.block_until_ready(r)
    import os
    pipeline = int(os.environ.get("KERNEL_PIPE", "1"))
    nin_ = len(in_names)
    best = float("inf")
    for _ in range(iters):
        t0 = time.perf_counter()
        for _k in range(pipeline):
            r = fn(*dargs[:nin_], *r)
        jax.block_until_ready(r)
        best = min(best, time.perf_counter() - t0)
    outs = [np.asarray(x) for x in r]
    per_core = np.split(outs[out_names.index("out_dev")], NCORES, axis=0)
    full_out = []
    for c in range(NCORES):
        pooled, wsum = decode_out(per_core[c], bcs)
        full_out.append(pooled / wsum[:, None])
    return best, np.concatenate(full_out, 0).astype(np.float32)


if __name__ == "__main__":
    rng = np.random.default_rng(0)
    ins = {
        "target_embedding": rng.standard_normal((B, D), dtype=np.float32),
        "hist_embeddings": rng.standard_normal((B, T, D), dtype=np.float32),
        "W_kernel": (rng.standard_normal((D, D), dtype=np.float32) / np.sqrt(D)),
        "W_bias": np.zeros(D, np.float32),
        "q_kernel": (rng.standard_normal((D, 1), dtype=np.float32) / np.sqrt(D)),
        "q_bias": np.zeros(1, np.float32),
    }
    out = kernel(**ins)
    print("out", out.shape, out.dtype)
